# revision 19
# baseline (speedup 1.0000x reference)
"""AttnBlock (GroupNorm + single-head HWxHW attention + residual) on 8 trn2 cores.

Sharding: data-parallel over (batch, query-half): core i handles batch i//2,
query columns [ (i%2)*2048, (i%2+1)*2048 ).  The input for odd cores is
column-rotated on the host so every core's queries are columns 0:2048 of its
input (softmax over keys is permutation invariant, so k/v order doesn't
matter) -- this keeps the program SPMD (one NEFF for all 8 cores).

Device algorithm (per core, C=128 channels on partitions, N=4096 spatial):
  - x streams in 4 chunks alternating both HWDGE rings; GroupNorm stats run
    chunk-wise on the DVE as data arrives, interleaved with bf16 casts of x.
  - rstd = exp(-0.5*ln(var+eps)) on the scalar engine: Ln/Exp/Copy share one
    activation table set, so the kernel does exactly one ACT table load,
    warmed during the DMA prologue.  No mid-pipeline table swap.
  - mu/rstd fold into bf16 copies of the weights on device, so q/k/v matmuls
    stream the bf16 x directly (no h tensor).  The -W'mu corrections: dropped
    for k (softmax-invariant), applied to q as a per-partition bias in the
    PSUM->SBUF copy, folded into an output-projection bias for v.
  - While waiting for x, the PE runs a train of dummy matmuls to ramp the
    DVFS p-state so the projection/score matmuls start at full clock.
  - Scores are computed transposed: sT[m-tile, n-block] = k_tile^T . q_blk in
    bf16 (fp32 PSUM).  exp() runs on the scalar engine over two PSUM banks at
    a time, writing bf16 (no row-max: scores are O(5), fp32-safe).
  - Softmax denominator: the first DEN_SPLIT[jb] exp groups accumulate on the
    DVE in bf16 (2x mode), the rest on the PE into a [128,512] PSUM tile via
    all-ones [128,128] matmuls (every output row = den), so the reciprocal is
    already broadcast across partitions -- no gpsimd broadcast.
  - PV: num[c, n] += vT_tile^T . pT_tile accumulated over 32 m-tiles in PSUM;
    the drain multiplies by 1/den; the output projection needs only one fused
    DVE op: out = (wp.(num/den) - obias) + x.
  - Block tails (den fold / reciprocal / PV drain / projection / store) are
    deferred into the NEXT block's early iterations so the in-order PE queue
    never stalls the next block's score matmuls; projections not needed for
    the first score group are spread as per-group fillers across block 0.

Host folding: gn_scale/gn_bias fold into the q/k/v weights and biases; the k
bias is dropped (softmax invariant); the v bias folds into the output bias.
"""

import os
import sys
import types

if "/opt/trn_rl_repo" not in sys.path:
    sys.path.insert(0, "/opt/trn_rl_repo")

import numpy as np

B, C, H, W = 4, 128, 64, 64
N = H * W              # 4096 spatial positions
NQ = N // 2            # 2048 queries per core
NB = 512               # query block (columns per psum bank)
NBLK = NQ // NB        # 4 query blocks
MT = N // 128          # 32 key tiles
NCH = 4                # x chunking (1024 columns per chunk)
CHW = N // NCH         # 1024
GROUPS = 8
GSIZE = C // GROUPS    # 16 channels per group
EPS = 1e-6
SCALE = float(C) ** -0.5
EXP_GRP = 2            # psum banks (512-wide matmuls) per exp instruction
NG = MT // EXP_GRP     # 16 exp groups per block
DEN_SPLIT = [10, 16, 16, 12]  # per block: exp groups accumulated on DVE
N_WARM = 34            # dummy matmuls to ramp the PE p-state during x DMA

LAST_RESULTS = None    # BassKernelResults of the most recent kernel() call


def _install_ntff_hook():
    """antenv.axon_hooks is missing from this container; inject it so
    run_bass_kernel_spmd(trace=True) can capture NTFF profiles."""
    if "antenv.axon_hooks" in sys.modules:
        return
    mod = types.ModuleType("antenv.axon_hooks")
    holder = [None]
    mod.set_axon_ntff_profile_hook = lambda h: holder.__setitem__(0, h)
    mod.get_axon_ntff_profile_hook = lambda: holder[0]
    sys.modules["antenv.axon_hooks"] = mod
    try:
        from trn_agent_boot.trn_boot import _ntff_profile_via_ctypes

        mod.set_axon_ntff_profile_hook(
            _ntff_profile_via_ctypes("/opt/axon/libaxon_pjrt.so")
        )
    except Exception:
        pass


_NC_CACHE = {}


def _build(use_bq: bool, use_bp: bool):
    key = (use_bq, use_bp)
    if key in _NC_CACHE:
        return _NC_CACHE[key]

    import concourse.bacc as bacc
    import concourse.mybir as mybir
    import concourse.tile as tile

    f32 = mybir.dt.float32
    bf16 = mybir.dt.bfloat16

    nc = bacc.Bacc("TRN2", target_bir_lowering=False, debug=False, num_devices=8)

    xp = nc.dram_tensor("xp", [C, N], f32, kind="ExternalInput")
    wqT_d = nc.dram_tensor("wqT", [C, C], f32, kind="ExternalInput")
    wkT_d = nc.dram_tensor("wkT", [C, C], f32, kind="ExternalInput")
    wvT_d = nc.dram_tensor("wvT", [C, C], f32, kind="ExternalInput")
    wpT_d = nc.dram_tensor("wpT", [C, C], f32, kind="ExternalInput")
    bq_d = nc.dram_tensor("bqe", [C, 1], f32, kind="ExternalInput")
    bp_d = nc.dram_tensor("bpe", [C, 1], f32, kind="ExternalInput")
    out_d = nc.dram_tensor("out", [C, NQ], f32, kind="ExternalOutput")

    # Group-mean reduction mask: symmetric block-diagonal matrix that
    # averages each group's 16 channels in a single matmul.
    gmm_np = np.zeros((C, C), np.float32)
    for ch in range(C):
        g0 = (ch // GSIZE) * GSIZE
        gmm_np[ch, g0 : g0 + GSIZE] = 1.0 / GSIZE
    gmm_d = nc.inline_tensor(gmm_np, "gmask")

    Exp = mybir.ActivationFunctionType.Exp
    Ln = mybir.ActivationFunctionType.Ln
    Copy = mybir.ActivationFunctionType.Copy
    add_op = mybir.AluOpType.add
    sub_op = mybir.AluOpType.subtract
    mult_op = mybir.AluOpType.mult

    with tile.TileContext(nc) as tc:
        with (
            tc.tile_pool(name="big", bufs=1) as big,
            tc.tile_pool(name="wgt", bufs=1) as wgt,
            tc.tile_pool(name="ptile", bufs=8) as ptile,
            tc.tile_pool(name="small", bufs=2) as small,
            tc.tile_pool(name="ostage", bufs=3) as ostage,
            tc.tile_pool(name="ps_s", bufs=2, space="PSUM") as ps_s,
            tc.tile_pool(name="ps_pv", bufs=2, space="PSUM") as ps_pv,
            tc.tile_pool(name="ps_den", bufs=1, space="PSUM") as ps_den,
            tc.tile_pool(name="ps_m", bufs=1, space="PSUM") as ps_m,
        ):
            # --- input DMAs: x chunks alternate both HWDGE rings (each ring
            # feeds its own DMA-engine subset), weights/masks follow x ---
            xc = []
            for j in range(NCH):
                xj = big.tile([C, CHW], f32, tag=f"x{j}")
                eng = nc.sync if j % 2 == 0 else nc.scalar
                eng.dma_start(out=xj[:], in_=xp.ap()[:, j * CHW : (j + 1) * CHW])
                xc.append(xj)
            gmm_sb = wgt.tile([C, C], f32, tag="gmm")
            nc.sync.dma_start(out=gmm_sb[:], in_=gmm_d.ap())
            w_q0 = wgt.tile([C, C], f32, tag="wq0")
            nc.scalar.dma_start(out=w_q0[:], in_=wqT_d.ap())
            w_k0 = wgt.tile([C, C], f32, tag="wk0")
            nc.sync.dma_start(out=w_k0[:], in_=wkT_d.ap())
            w_v0 = wgt.tile([C, C], f32, tag="wv0")
            nc.scalar.dma_start(out=w_v0[:], in_=wvT_d.ap())
            w_p0 = wgt.tile([C, C], f32, tag="wp0")
            nc.sync.dma_start(out=w_p0[:], in_=wpT_d.ap())
            if use_bq:
                bqe_sb = wgt.tile([C, 1], f32, tag="bqe")
                nc.scalar.dma_start(out=bqe_sb[:], in_=bq_d.ap())
            if use_bp:
                bpe_sb = wgt.tile([C, 1], f32, tag="bpe")
                nc.scalar.dma_start(out=bpe_sb[:], in_=bp_d.ap())

            eps_sb = wgt.tile([C, 1], f32, tag="eps")
            nc.vector.memset(eps_sb[:], EPS)
            ones_bf = wgt.tile([C, C], bf16, tag="ones")
            nc.gpsimd.memset(ones_bf[:], 1.0)
            # dummy Exp THEN Ln settle the ACT table on the combined
            # ln/exp/copy set during the DMA prologue, so no mid-pipeline
            # table swap ever happens
            warm = wgt.tile([1, 1], f32, tag="warm")
            nc.scalar.activation(out=warm[:], in_=eps_sb[0:1, :], func=Exp)
            nc.scalar.activation(out=warm[:], in_=eps_sb[0:1, :], func=Ln)

            # --- GroupNorm statistics + bf16 x casts (DVE, chunk-wise) ---
            stats = small.tile([C, 8, 6], f32, tag="stats")
            xb = []
            for j in range(NCH):
                for s in range(2):
                    nc.vector.bn_stats(
                        out=stats[:, 2 * j + s, :],
                        in_=xc[j][:, s * 512 : s * 512 + 512],
                    )
                if j < 3:
                    xbj = big.tile([C, CHW], bf16, tag=f"xb{j}")
                    nc.vector.tensor_copy(out=xbj[:], in_=xc[j][:])
                    xb.append(xbj)
            xb3 = big.tile([C, CHW], bf16, tag="xb3")  # cast deferred
            xb.append(xb3)

            # PE p-state warm-up: dummy matmuls into a throwaway psum bank
            # while the tail x chunks stream in (consumes nothing downstream)
            ps_warm = ps_m.tile([C, NB], f32, tag="m")
            for _ in range(N_WARM):
                nc.tensor.matmul(
                    ps_warm[:], lhsT=ones_bf[:], rhs=xb[0][:, 0:NB],
                    start=True, stop=True,
                )

            mv = small.tile([C, 2], f32, tag="mv")
            nc.vector.bn_aggr(out=mv[:], in_=stats[:])
            # t2 = per-channel [sum(x), sum(x^2)]; gmm averages the group
            t2 = small.tile([C, 2], f32, tag="t2")
            nc.vector.tensor_copy(out=t2[:, 0:1], in_=mv[:, 0:1])
            nc.vector.tensor_tensor(t2[:, 1:2], mv[:, 0:1], mv[:, 0:1], mult_op)
            nc.vector.tensor_tensor(t2[:, 1:2], t2[:, 1:2], mv[:, 1:2], add_op)
            psb = ps_m.tile([C, 2], f32, tag="m")
            nc.tensor.matmul(psb[:], lhsT=gmm_sb[:], rhs=t2[:], start=True, stop=True)
            # mu = E[x]; var = E[x^2] - mu^2 ; rstd = exp(-0.5*ln(var+eps))
            mu = small.tile([C, 1], f32, tag="mu")
            nc.vector.tensor_copy(out=mu[:], in_=psb[:, 0:1])
            var = small.tile([C, 1], f32, tag="var")
            nc.vector.tensor_tensor(var[:], mu[:], mu[:], mult_op)
            nc.vector.tensor_tensor(var[:], psb[:, 1:2], var[:], sub_op)
            lnv = small.tile([C, 1], f32, tag="lnv")
            nc.scalar.activation(out=lnv[:], in_=var[:], func=Ln, bias=eps_sb[:])
            rstd = small.tile([C, 1], f32, tag="rstd")
            nc.scalar.activation(out=rstd[:], in_=lnv[:], func=Exp, scale=-0.5)
            # wp/mub need no rstd: emitted while ACT computes it
            mub = small.tile([C, 1], bf16, tag="mub")
            nc.vector.tensor_copy(out=mub[:], in_=mu[:])
            w_p = wgt.tile([C, C], bf16, tag="wp")
            nc.vector.tensor_copy(out=w_p[:], in_=w_p0[:])

            # fold rstd into the bf16 weights (per-in-channel scale)
            w_q = wgt.tile([C, C], bf16, tag="wq")
            nc.vector.tensor_scalar(w_q[:], w_q0[:], rstd[:], None, op0=mult_op)
            w_k = wgt.tile([C, C], bf16, tag="wk")
            nc.vector.tensor_scalar(w_k[:], w_k0[:], rstd[:], None, op0=mult_op)
            w_v = wgt.tile([C, C], bf16, tag="wv")
            nc.vector.tensor_scalar(w_v[:], w_v0[:], rstd[:], None, op0=mult_op)

            def xbpart(lo, width):
                j = lo // CHW
                assert lo + width <= (j + 1) * CHW
                return xb[j][:, lo - j * CHW : lo - j * CHW + width]

            kc = [None] * 8

            def kpart(mi):
                return kc[mi // 4][:, (mi % 4) * 128 : (mi % 4) * 128 + 128]

            qb = [None] * NBLK
            vT_sb = big.tile([128, MT, C], bf16, tag="vt")
            bqs = small.tile([C, 1], f32, tag="bqs")
            bvb = small.tile([C, 1], bf16, tag="bvb")
            obs = small.tile([C, 1], f32, tag="obs")

            # --- critical-path projections: q0 and k slice 0 via ps_s ---
            psq0 = ps_s.tile([C, EXP_GRP, NB], f32, tag="s", name="psq0")
            nc.tensor.matmul(
                psq0[:, 0, :], lhsT=w_q[:], rhs=xbpart(0, NB), start=True, stop=True
            )
            # q-bias correction -Wq'.mu (reuses the wq stationary)
            psbq = ps_m.tile([C, 1], f32, tag="m")
            nc.tensor.matmul(psbq[:], lhsT=w_q[:], rhs=mub[:], start=True, stop=True)
            psk0 = ps_s.tile([C, EXP_GRP, NB], f32, tag="s", name="psk0")
            nc.tensor.matmul(
                psk0[:, 0, :], lhsT=w_k[:], rhs=xbpart(0, NB), start=True, stop=True
            )
            # v-bias chain part 1: Wv'.mu
            psbv = ps_m.tile([C, 1], f32, tag="m")
            nc.tensor.matmul(psbv[:], lhsT=w_v[:], rhs=mub[:], start=True, stop=True)

            nc.vector.tensor_copy(out=bqs[:], in_=psbq[:])
            q0 = big.tile([C, NB], bf16, tag="q0")
            if use_bq:
                nc.vector.tensor_scalar(
                    q0[:], psq0[:, 0, :], bqs[:], bqe_sb[:], op0=sub_op, op1=add_op
                )
            else:
                nc.vector.tensor_scalar(q0[:], psq0[:, 0, :], bqs[:], None, op0=sub_op)
            qb[0] = q0
            # k slice 0 on the scalar engine (idle until the first exp) so it
            # runs parallel to the q0 copy on DVE
            k0t = big.tile([C, NB], bf16, tag="k0")
            nc.scalar.activation(out=k0t[:], in_=psk0[:, 0, :], func=Copy)
            kc[0] = k0t
            nc.vector.tensor_copy(out=bvb[:], in_=psbv[:])

            # --- filler emitters (interleaved projections) ---
            def emit_k(s):
                psk = ps_m.tile([C, NB], f32, tag="m", name=f"psk{s}")
                nc.tensor.matmul(
                    psk[:], lhsT=w_k[:], rhs=xbpart(s * NB, NB), start=True, stop=True
                )
                kj = big.tile([C, NB], bf16, tag=f"k{s}")
                nc.vector.tensor_copy(out=kj[:], in_=psk[:])
                kc[s] = kj

            def emit_q(jb):
                psq = ps_m.tile([C, NB], f32, tag="m", name=f"psq{jb}")
                nc.tensor.matmul(
                    psq[:], lhsT=w_q[:], rhs=xbpart(jb * NB, NB), start=True, stop=True
                )
                qj = big.tile([C, NB], bf16, tag=f"q{jb}")
                if use_bq:
                    nc.vector.tensor_scalar(
                        qj[:], psq[:], bqs[:], bqe_sb[:], op0=sub_op, op1=add_op
                    )
                else:
                    nc.vector.tensor_scalar(qj[:], psq[:], bqs[:], None, op0=sub_op)
                qb[jb] = qj

            def emit_vb(b, eng=None):
                # 4 transposed v tiles into one psum bank, one DVE drain
                psv = ps_m.tile([128, 4, C], f32, tag="m", name=f"psv{b}")
                for t in range(4):
                    mi = 4 * b + t
                    nc.tensor.matmul(
                        psv[:, t, :],
                        lhsT=xbpart(mi * 128, 128),
                        rhs=w_v[:],
                        start=True,
                        stop=True,
                    )
                (eng or nc.vector).tensor_copy(
                    out=vT_sb[:, 4 * b : 4 * b + 4, :], in_=psv[:]
                )

            def emit_xb3():
                # SBUF->SBUF cast on the otherwise-idle gpsimd
                nc.gpsimd.tensor_copy(out=xb3[:], in_=xc[3][:])

            def emit_psob():
                # v-bias chain part 2: obias = Wp.(Wv'.mu)  [minus bpe if any]
                psob = ps_m.tile([C, 1], f32, tag="m")
                nc.tensor.matmul(psob[:], lhsT=w_p[:], rhs=bvb[:], start=True, stop=True)
                if use_bp:
                    nc.vector.tensor_tensor(obs[:], psob[:], bpe_sb[:], sub_op)
                else:
                    nc.vector.tensor_copy(out=obs[:], in_=psob[:])

            # pre-loop fillers (ps_m FIFO paces one copy per slot)
            emit_k(1)
            emit_k(2)
            emit_vb(0)

            # filler schedule: (block, group) -> emitter.  k slice s must land
            # before score group 2s; v batch b before PV step 2b+1; q_jb
            # before block jb; psob before block 0's tail (run at block1 g4).
            fillers = {
                (0, 0): lambda: emit_k(3),
                (0, 1): lambda: (emit_vb(1), emit_xb3()),
                (0, 2): lambda: emit_k(4),
                (0, 3): lambda: emit_vb(2),
                (0, 4): lambda: emit_vb(3),
                (0, 5): lambda: emit_vb(4),
                (0, 6): lambda: emit_k(5),
                (0, 7): lambda: emit_vb(5),
                (0, 8): lambda: emit_q(1),
                (0, 9): lambda: emit_vb(6),
                (0, 10): lambda: emit_k(6),
                (0, 11): lambda: emit_vb(7),
                (0, 12): lambda: emit_k(7),
                (1, 0): emit_psob,
                (1, 1): lambda: emit_q(2),
                (1, 3): lambda: emit_q(3),
            }

            # deferred tail state: (pv, dn, dacc_fold, jb) of the previous block
            pend = [None]

            def tail_a():
                # den fold matmul + reciprocal of the previous block; if the
                # whole denominator accumulated on DVE this is dn's only
                # matmul and must open the psum accumulation group
                pv_p, dn_p, dfold_p, jb_p, den_g_p = pend[0]
                nc.tensor.matmul(
                    dn_p[:], lhsT=ones_bf[:], rhs=dfold_p[:],
                    start=(den_g_p == NG), stop=True,
                )
                rden = ostage.tile([128, NB], f32, tag="rden")
                nc.vector.reciprocal_approx_fast(out=rden[:], in_=dn_p[:])
                pend[0] = (pv_p, rden, jb_p)

            def tail_b(last=False):
                # PV drain, output projection, residual, store.  For the last
                # block the normalization moves AFTER the projection (divide
                # by den commutes with the channel matmul), so the PV drain
                # and projection overlap the denominator reciprocal.
                pv_p, rden, jb_p = pend[0]
                pend[0] = None
                hv = ostage.tile([C, NB], bf16, tag="hv")
                if last:
                    nc.vector.tensor_copy(out=hv[:], in_=pv_p[:])
                else:
                    nc.vector.tensor_tensor(hv[:], pv_p[:], rden[:], mult_op)
                pso = ps_m.tile([C, NB], f32, tag="m")
                nc.tensor.matmul(pso[:], lhsT=w_p[:], rhs=hv[:], start=True, stop=True)
                o1 = ostage.tile([C, NB], f32, tag="o1")
                xblk = xc[jb_p // 2][:, (jb_p % 2) * 512 : (jb_p % 2) * 512 + 512]
                if last:
                    on = ostage.tile([C, NB], f32, tag="on")
                    nc.vector.tensor_tensor(on[:], pso[:], rden[:], mult_op)
                    nc.vector.scalar_tensor_tensor(
                        o1[:], on[:], obs[:], xblk, op0=sub_op, op1=add_op
                    )
                else:
                    nc.vector.scalar_tensor_tensor(
                        o1[:], pso[:], obs[:], xblk, op0=sub_op, op1=add_op
                    )
                nc.sync.dma_start(
                    out=out_d[:, jb_p * NB : (jb_p + 1) * NB], in_=o1[:]
                )

            # --- attention over query blocks ---
            for jb in range(NBLK):
                den_g = DEN_SPLIT[jb]
                qs = qb[jb][:]
                pv = ps_pv.tile([C, NB], f32, tag="pv")
                dn = ps_den.tile([C, NB], f32, tag="dn")
                dacc = ostage.tile([128, EXP_GRP, NB], bf16, tag="dacc")
                dfold = ostage.tile([128, NB], bf16, tag="dfold")
                pts = [None] * NG
                # software-pipelined by one group: scores/exp for g are
                # emitted ahead of group g-1's consumers so the scalar engine
                # never starves behind PV/den matmuls.
                for g in range(NG + 1):
                    if g < NG:
                        ss = ps_s.tile([128, EXP_GRP, NB], f32, tag="s")
                        for u in range(EXP_GRP):
                            mi = g * EXP_GRP + u
                            nc.tensor.matmul(
                                ss[:, u, :],
                                lhsT=kpart(mi),
                                rhs=qs,
                                start=True,
                                stop=True,
                            )
                        pt = ptile.tile([128, EXP_GRP, NB], bf16, tag="pt")
                        nc.scalar.activation(
                            out=pt[:], in_=ss[:], func=Exp, scale=SCALE
                        )
                        pts[g] = pt
                        fill = fillers.get((jb, g))
                        if fill is not None:
                            fill()
                    if pend[0] is not None:
                        if g == 2:
                            tail_a()
                        elif g == 4:
                            tail_b()
                    if g == 0:
                        continue
                    c = g - 1
                    pt = pts[c]
                    pts[c] = None
                    for u in range(EXP_GRP):
                        mi = c * EXP_GRP + u
                        nc.tensor.matmul(
                            pv[:],
                            lhsT=vT_sb[:, mi, :],
                            rhs=pt[:, u, :],
                            start=(mi == 0),
                            stop=(mi == MT - 1),
                        )
                    if c < den_g:
                        # denominator partial on DVE (bf16 SBUF adds, 2x mode)
                        if c == 0:
                            nc.vector.tensor_copy(out=dacc[:], in_=pt[:])
                        else:
                            nc.vector.tensor_tensor(dacc[:], dacc[:], pt[:], add_op)
                        if c == den_g - 1:
                            nc.vector.tensor_tensor(
                                dfold[:], dacc[:, 0, :], dacc[:, 1, :], add_op
                            )
                    else:
                        # denominator partial on PE; the all-ones stationary
                        # writes den into every output partition, so the
                        # reciprocal needs no cross-partition broadcast
                        for u in range(EXP_GRP):
                            mi = c * EXP_GRP + u
                            nc.tensor.matmul(
                                dn[:],
                                lhsT=ones_bf[:],
                                rhs=pt[:, u, :],
                                start=(c == den_g and u == 0),
                                stop=False,
                            )
                pend[0] = (pv, dn, dfold, jb, den_g)
            # last block's tail runs immediately
            tail_a()
            tail_b(last=True)

    nc.compile()
    _NC_CACHE[key] = nc
    return nc


def kernel(**inputs):
    global LAST_RESULTS
    _install_ntff_hook()
    from concourse.bass_utils import run_bass_kernel_spmd

    ins = {
        k: np.ascontiguousarray(np.asarray(v), dtype=np.float32)
        for k, v in inputs.items()
    }
    x = ins["x"]
    gs, gb = ins["gn_scale"], ins["gn_bias"]

    # Fold the GroupNorm affine into the q/k/v weights; pre-transpose all
    # weights into the [in_channel, out_channel] layout the PE wants.
    wq_e = ins["wq"] * gs[None, :]
    wk_e = ins["wk"] * gs[None, :]
    wv_e = ins["wv"] * gs[None, :]
    wqT = np.ascontiguousarray(wq_e.T)
    wkT = np.ascontiguousarray(wk_e.T)
    wvT = np.ascontiguousarray(wv_e.T)
    wpT = np.ascontiguousarray(ins["wp"].T)
    bq_e = (ins["bq"] + ins["wq"] @ gb).reshape(C, 1)
    bv_e = ins["bv"] + ins["wv"] @ gb
    bp_e = (ins["bp"] + ins["wp"] @ bv_e).reshape(C, 1)
    use_bq = bool(np.any(bq_e))
    use_bp = bool(np.any(bp_e))

    nc = _build(use_bq, use_bp)

    in_maps = []
    for core in range(8):
        b, half = core // 2, core % 2
        xb = x[b].reshape(C, N)
        if half == 1:
            xb = np.concatenate([xb[:, NQ:], xb[:, :NQ]], axis=1)
        in_maps.append(
            {
                "xp": np.ascontiguousarray(xb),
                "wqT": wqT,
                "wkT": wkT,
                "wvT": wvT,
                "wpT": wpT,
                "bqe": bq_e,
                "bpe": bp_e,
            }
        )

    trace = os.environ.get("KERNEL_TRACE", "0") == "1"
    res = run_bass_kernel_spmd(nc, in_maps, core_ids=list(range(8)), trace=trace)
    LAST_RESULTS = res

    out = np.empty((B, C, N), np.float32)
    for core in range(8):
        b, half = core // 2, core % 2
        out[b, :, half * NQ : (half + 1) * NQ] = res.results[core]["out"]
    return out.reshape(B, C, H, W)


# revision 22
# speedup vs baseline: 1.0302x; 1.0302x over previous
"""AttnBlock (GroupNorm + single-head HWxHW attention + residual) on 8 trn2 cores.

Sharding: data-parallel over (batch, query-half): core i handles batch i//2,
query columns [ (i%2)*2048, (i%2+1)*2048 ).  The input for odd cores is
column-rotated on the host so every core's queries are columns 0:2048 of its
input (softmax over keys is permutation invariant, so k/v order doesn't
matter) -- this keeps the program SPMD (one NEFF for all 8 cores).

Device algorithm (per core, C=128 channels on partitions, N=4096 spatial):
  - x streams in 4 chunks alternating both HWDGE rings; GroupNorm stats run
    chunk-wise on the DVE as data arrives, interleaved with bf16 casts of x.
  - rstd = exp(-0.5*ln(var+eps)) on the scalar engine: Ln/Exp/Copy share one
    activation table set, so the kernel does exactly one ACT table load,
    warmed during the DMA prologue.  No mid-pipeline table swap.
  - mu/rstd fold into bf16 copies of the weights on device, so q/k/v matmuls
    stream the bf16 x directly (no h tensor).  The -W'mu corrections: dropped
    for k (softmax-invariant), applied to q as a per-partition bias in the
    PSUM->SBUF copy, folded into an output-projection bias for v.
  - While waiting for x, the PE runs a train of dummy matmuls to ramp the
    DVFS p-state so the projection/score matmuls start at full clock.
  - Scores are computed transposed: sT[m-tile, n-block] = k_tile^T . q_blk in
    bf16 (fp32 PSUM).  exp() runs on the scalar engine over two PSUM banks at
    a time, writing bf16 (no row-max: scores are O(5), fp32-safe).
  - Softmax denominator: the first DEN_SPLIT[jb] exp groups accumulate on the
    DVE in bf16 (2x mode), the rest on the PE into a [128,512] PSUM tile via
    all-ones [128,128] matmuls (every output row = den), so the reciprocal is
    already broadcast across partitions -- no gpsimd broadcast.
  - PV: num[c, n] += vT_tile^T . pT_tile accumulated over 32 m-tiles in PSUM;
    the drain multiplies by 1/den; the output projection needs only one fused
    DVE op: out = (wp.(num/den) - obias) + x.
  - Block tails (den fold / reciprocal / PV drain / projection / store) are
    deferred into the NEXT block's early iterations so the in-order PE queue
    never stalls the next block's score matmuls; projections not needed for
    the first score group are spread as per-group fillers across block 0.

Host folding: gn_scale/gn_bias fold into the q/k/v weights and biases; the k
bias is dropped (softmax invariant); the v bias folds into the output bias.
"""

import os
import sys
import types

if "/opt/trn_rl_repo" not in sys.path:
    sys.path.insert(0, "/opt/trn_rl_repo")

import numpy as np

B, C, H, W = 4, 128, 64, 64
N = H * W              # 4096 spatial positions
NQ = N // 2            # 2048 queries per core
NB = 512               # query block (columns per psum bank)
NBLK = NQ // NB        # 4 query blocks
MT = N // 128          # 32 key tiles
NCH = 4                # x chunking (1024 columns per chunk)
CHW = N // NCH         # 1024
GROUPS = 8
GSIZE = C // GROUPS    # 16 channels per group
EPS = 1e-6
SCALE = float(C) ** -0.5
EXP_GRP = 2            # psum banks (512-wide matmuls) per exp instruction
NG = MT // EXP_GRP     # 16 exp groups per block
DEN_SPLIT = [10, 16, 16, 12]  # per block: exp groups accumulated on DVE
N_WARM = 34            # dummy matmuls to ramp the PE p-state during x DMA

LAST_RESULTS = None    # BassKernelResults of the most recent kernel() call


def _install_ntff_hook():
    """antenv.axon_hooks is missing from this container; inject it so
    run_bass_kernel_spmd(trace=True) can capture NTFF profiles."""
    if "antenv.axon_hooks" in sys.modules:
        return
    mod = types.ModuleType("antenv.axon_hooks")
    holder = [None]
    mod.set_axon_ntff_profile_hook = lambda h: holder.__setitem__(0, h)
    mod.get_axon_ntff_profile_hook = lambda: holder[0]
    sys.modules["antenv.axon_hooks"] = mod
    try:
        from trn_agent_boot.trn_boot import _ntff_profile_via_ctypes

        mod.set_axon_ntff_profile_hook(
            _ntff_profile_via_ctypes("/opt/axon/libaxon_pjrt.so")
        )
    except Exception:
        pass


_NC_CACHE = {}


def _build(use_bq: bool, use_bp: bool):
    key = (use_bq, use_bp)
    if key in _NC_CACHE:
        return _NC_CACHE[key]

    import concourse.bacc as bacc
    import concourse.hw_specs as hw_specs
    import concourse.mybir as mybir
    import concourse.tile as tile

    f32 = mybir.dt.float32
    bf16 = mybir.dt.bfloat16

    nc = bacc.Bacc("TRN2", target_bir_lowering=False, debug=False, num_devices=8)

    # This kernel's only table-based activations are Ln and Exp.  The table
    # chooser takes the first set containing the requested function, which
    # puts ln and exp in different sets and forces a 1.3us table swap right
    # inside the rstd critical path.  Narrow the compile-time coverage view
    # so both resolve to the one set that holds ln AND exp (runtime table
    # contents come from act_info.json and are unaffected; set ids keep
    # their original positions).
    _tabs = hw_specs.get_activation_tables(nc.m.arch)
    _Exp = mybir.ActivationFunctionType.Exp
    _Ln = mybir.ActivationFunctionType.Ln
    if "natural_log_exp_and_others" in _tabs:
        for _name, _funcs in _tabs.items():
            if _name != "natural_log_exp_and_others":
                _funcs.discard(_Exp)
                _funcs.discard(_Ln)

    xp = nc.dram_tensor("xp", [C, N], f32, kind="ExternalInput")
    wqT_d = nc.dram_tensor("wqT", [C, C], f32, kind="ExternalInput")
    wkT_d = nc.dram_tensor("wkT", [C, C], f32, kind="ExternalInput")
    wvT_d = nc.dram_tensor("wvT", [C, C], f32, kind="ExternalInput")
    wpT_d = nc.dram_tensor("wpT", [C, C], f32, kind="ExternalInput")
    bq_d = nc.dram_tensor("bqe", [C, 1], f32, kind="ExternalInput")
    bp_d = nc.dram_tensor("bpe", [C, 1], f32, kind="ExternalInput")
    out_d = nc.dram_tensor("out", [C, NQ], f32, kind="ExternalOutput")

    # Group-mean reduction mask: symmetric block-diagonal matrix that
    # averages each group's 16 channels in a single matmul.
    gmm_np = np.zeros((C, C), np.float32)
    for ch in range(C):
        g0 = (ch // GSIZE) * GSIZE
        gmm_np[ch, g0 : g0 + GSIZE] = 1.0 / GSIZE
    gmm_d = nc.inline_tensor(gmm_np, "gmask")

    Exp = mybir.ActivationFunctionType.Exp
    Ln = mybir.ActivationFunctionType.Ln
    Copy = mybir.ActivationFunctionType.Copy
    add_op = mybir.AluOpType.add
    sub_op = mybir.AluOpType.subtract
    mult_op = mybir.AluOpType.mult

    with tile.TileContext(nc) as tc:
        with (
            tc.tile_pool(name="big", bufs=1) as big,
            tc.tile_pool(name="wgt", bufs=1) as wgt,
            tc.tile_pool(name="ptile", bufs=8) as ptile,
            tc.tile_pool(name="small", bufs=2) as small,
            tc.tile_pool(name="ostage", bufs=3) as ostage,
            tc.tile_pool(name="ps_s", bufs=2, space="PSUM") as ps_s,
            tc.tile_pool(name="ps_pv", bufs=2, space="PSUM") as ps_pv,
            tc.tile_pool(name="ps_den", bufs=1, space="PSUM") as ps_den,
            tc.tile_pool(name="ps_m", bufs=1, space="PSUM") as ps_m,
        ):
            # --- input DMAs: x chunks alternate both HWDGE rings (each ring
            # feeds its own DMA-engine subset), weights/masks follow x ---
            xc = []
            for j in range(NCH):
                xj = big.tile([C, CHW], f32, tag=f"x{j}")
                eng = nc.sync if j % 2 == 0 else nc.scalar
                eng.dma_start(out=xj[:], in_=xp.ap()[:, j * CHW : (j + 1) * CHW])
                xc.append(xj)
            gmm_sb = wgt.tile([C, C], f32, tag="gmm")
            nc.sync.dma_start(out=gmm_sb[:], in_=gmm_d.ap())
            w_q0 = wgt.tile([C, C], f32, tag="wq0")
            nc.scalar.dma_start(out=w_q0[:], in_=wqT_d.ap())
            w_k0 = wgt.tile([C, C], f32, tag="wk0")
            nc.sync.dma_start(out=w_k0[:], in_=wkT_d.ap())
            w_v0 = wgt.tile([C, C], f32, tag="wv0")
            nc.scalar.dma_start(out=w_v0[:], in_=wvT_d.ap())
            w_p0 = wgt.tile([C, C], f32, tag="wp0")
            nc.sync.dma_start(out=w_p0[:], in_=wpT_d.ap())
            if use_bq:
                bqe_sb = wgt.tile([C, 1], f32, tag="bqe")
                nc.scalar.dma_start(out=bqe_sb[:], in_=bq_d.ap())
            if use_bp:
                bpe_sb = wgt.tile([C, 1], f32, tag="bpe")
                nc.scalar.dma_start(out=bpe_sb[:], in_=bp_d.ap())

            eps_sb = wgt.tile([C, 1], f32, tag="eps")
            nc.vector.memset(eps_sb[:], EPS)
            ones_bf = wgt.tile([C, C], bf16, tag="ones")
            nc.gpsimd.memset(ones_bf[:], 1.0)
            # dummy Exp THEN Ln settle the ACT table on the combined
            # ln/exp/copy set during the DMA prologue, so no mid-pipeline
            # table swap ever happens
            warm = wgt.tile([1, 1], f32, tag="warm")
            nc.scalar.activation(out=warm[:], in_=eps_sb[0:1, :], func=Exp)
            nc.scalar.activation(out=warm[:], in_=eps_sb[0:1, :], func=Ln)

            # --- GroupNorm statistics + bf16 x casts (DVE, chunk-wise) ---
            stats = small.tile([C, 8, 6], f32, tag="stats")
            xb = []
            for j in range(NCH):
                for s in range(2):
                    nc.vector.bn_stats(
                        out=stats[:, 2 * j + s, :],
                        in_=xc[j][:, s * 512 : s * 512 + 512],
                    )
                if j < 3:
                    xbj = big.tile([C, CHW], bf16, tag=f"xb{j}")
                    nc.vector.tensor_copy(out=xbj[:], in_=xc[j][:])
                    xb.append(xbj)
            xb3 = big.tile([C, CHW], bf16, tag="xb3")  # cast deferred
            xb.append(xb3)

            # PE p-state warm-up: dummy matmuls into a throwaway psum bank
            # while the tail x chunks stream in (consumes nothing downstream)
            ps_warm = ps_m.tile([C, NB], f32, tag="m")
            for _ in range(N_WARM):
                nc.tensor.matmul(
                    ps_warm[:], lhsT=ones_bf[:], rhs=xb[0][:, 0:NB],
                    start=True, stop=True,
                )

            mv = small.tile([C, 2], f32, tag="mv")
            nc.vector.bn_aggr(out=mv[:], in_=stats[:])
            # t2 = per-channel [sum(x), sum(x^2)]; gmm averages the group
            t2 = small.tile([C, 2], f32, tag="t2")
            nc.vector.tensor_copy(out=t2[:, 0:1], in_=mv[:, 0:1])
            nc.vector.tensor_tensor(t2[:, 1:2], mv[:, 0:1], mv[:, 0:1], mult_op)
            nc.vector.tensor_tensor(t2[:, 1:2], t2[:, 1:2], mv[:, 1:2], add_op)
            psb = ps_m.tile([C, 2], f32, tag="m")
            nc.tensor.matmul(psb[:], lhsT=gmm_sb[:], rhs=t2[:], start=True, stop=True)
            # mu = E[x]; var = E[x^2] - mu^2 ; rstd = exp(-0.5*ln(var+eps))
            mu = small.tile([C, 1], f32, tag="mu")
            nc.vector.tensor_copy(out=mu[:], in_=psb[:, 0:1])
            var = small.tile([C, 1], f32, tag="var")
            nc.vector.tensor_tensor(var[:], mu[:], mu[:], mult_op)
            nc.vector.tensor_tensor(var[:], psb[:, 1:2], var[:], sub_op)
            lnv = small.tile([C, 1], f32, tag="lnv")
            nc.scalar.activation(out=lnv[:], in_=var[:], func=Ln, bias=eps_sb[:])
            rstd = small.tile([C, 1], f32, tag="rstd")
            nc.scalar.activation(out=rstd[:], in_=lnv[:], func=Exp, scale=-0.5)
            # wp/mub need no rstd: emitted while ACT computes it
            mub = small.tile([C, 1], bf16, tag="mub")
            nc.vector.tensor_copy(out=mub[:], in_=mu[:])
            w_p = wgt.tile([C, C], bf16, tag="wp")
            nc.vector.tensor_copy(out=w_p[:], in_=w_p0[:])

            # fold rstd into the bf16 weights (per-in-channel scale)
            w_q = wgt.tile([C, C], bf16, tag="wq")
            nc.vector.tensor_scalar(w_q[:], w_q0[:], rstd[:], None, op0=mult_op)
            w_k = wgt.tile([C, C], bf16, tag="wk")
            nc.vector.tensor_scalar(w_k[:], w_k0[:], rstd[:], None, op0=mult_op)
            w_v = wgt.tile([C, C], bf16, tag="wv")
            nc.vector.tensor_scalar(w_v[:], w_v0[:], rstd[:], None, op0=mult_op)

            def xbpart(lo, width):
                j = lo // CHW
                assert lo + width <= (j + 1) * CHW
                return xb[j][:, lo - j * CHW : lo - j * CHW + width]

            kc = [None] * 8

            def kpart(mi):
                return kc[mi // 4][:, (mi % 4) * 128 : (mi % 4) * 128 + 128]

            qb = [None] * NBLK
            vT_sb = big.tile([128, MT, C], bf16, tag="vt")
            bqs = small.tile([C, 1], f32, tag="bqs")
            bvb = small.tile([C, 1], bf16, tag="bvb")
            obs = small.tile([C, 1], f32, tag="obs")

            # --- critical-path projections: q0 and k slice 0 via ps_s ---
            psq0 = ps_s.tile([C, EXP_GRP, NB], f32, tag="s", name="psq0")
            nc.tensor.matmul(
                psq0[:, 0, :], lhsT=w_q[:], rhs=xbpart(0, NB), start=True, stop=True
            )
            # q-bias correction -Wq'.mu (reuses the wq stationary)
            psbq = ps_m.tile([C, 1], f32, tag="m")
            nc.tensor.matmul(psbq[:], lhsT=w_q[:], rhs=mub[:], start=True, stop=True)
            psk0 = ps_s.tile([C, EXP_GRP, NB], f32, tag="s", name="psk0")
            nc.tensor.matmul(
                psk0[:, 0, :], lhsT=w_k[:], rhs=xbpart(0, NB), start=True, stop=True
            )
            # v-bias chain part 1: Wv'.mu
            psbv = ps_m.tile([C, 1], f32, tag="m")
            nc.tensor.matmul(psbv[:], lhsT=w_v[:], rhs=mub[:], start=True, stop=True)

            nc.vector.tensor_copy(out=bqs[:], in_=psbq[:])
            q0 = big.tile([C, NB], bf16, tag="q0")
            if use_bq:
                nc.vector.tensor_scalar(
                    q0[:], psq0[:, 0, :], bqs[:], bqe_sb[:], op0=sub_op, op1=add_op
                )
            else:
                nc.vector.tensor_scalar(q0[:], psq0[:, 0, :], bqs[:], None, op0=sub_op)
            qb[0] = q0
            # k slice 0 on the scalar engine (idle until the first exp) so it
            # runs parallel to the q0 copy on DVE
            k0t = big.tile([C, NB], bf16, tag="k0")
            nc.scalar.activation(out=k0t[:], in_=psk0[:, 0, :], func=Copy)
            kc[0] = k0t
            nc.vector.tensor_copy(out=bvb[:], in_=psbv[:])

            # --- filler emitters (interleaved projections) ---
            def emit_k(s):
                psk = ps_m.tile([C, NB], f32, tag="m", name=f"psk{s}")
                nc.tensor.matmul(
                    psk[:], lhsT=w_k[:], rhs=xbpart(s * NB, NB), start=True, stop=True
                )
                kj = big.tile([C, NB], bf16, tag=f"k{s}")
                nc.vector.tensor_copy(out=kj[:], in_=psk[:])
                kc[s] = kj

            def emit_q(jb):
                psq = ps_m.tile([C, NB], f32, tag="m", name=f"psq{jb}")
                nc.tensor.matmul(
                    psq[:], lhsT=w_q[:], rhs=xbpart(jb * NB, NB), start=True, stop=True
                )
                qj = big.tile([C, NB], bf16, tag=f"q{jb}")
                if use_bq:
                    nc.vector.tensor_scalar(
                        qj[:], psq[:], bqs[:], bqe_sb[:], op0=sub_op, op1=add_op
                    )
                else:
                    nc.vector.tensor_scalar(qj[:], psq[:], bqs[:], None, op0=sub_op)
                qb[jb] = qj

            def emit_vb(b, eng=None):
                # 4 transposed v tiles into one psum bank, one DVE drain
                psv = ps_m.tile([128, 4, C], f32, tag="m", name=f"psv{b}")
                for t in range(4):
                    mi = 4 * b + t
                    nc.tensor.matmul(
                        psv[:, t, :],
                        lhsT=xbpart(mi * 128, 128),
                        rhs=w_v[:],
                        start=True,
                        stop=True,
                    )
                (eng or nc.vector).tensor_copy(
                    out=vT_sb[:, 4 * b : 4 * b + 4, :], in_=psv[:]
                )

            def emit_xb3():
                nc.vector.tensor_copy(out=xb3[:], in_=xc[3][:])

            def emit_psob():
                # v-bias chain part 2: obias = Wp.(Wv'.mu)  [minus bpe if any]
                psob = ps_m.tile([C, 1], f32, tag="m")
                nc.tensor.matmul(psob[:], lhsT=w_p[:], rhs=bvb[:], start=True, stop=True)
                if use_bp:
                    nc.vector.tensor_tensor(obs[:], psob[:], bpe_sb[:], sub_op)
                else:
                    nc.vector.tensor_copy(out=obs[:], in_=psob[:])

            # pre-loop fillers (ps_m FIFO paces one copy per slot)
            emit_k(1)
            emit_k(2)
            emit_k(3)
            emit_vb(0)
            emit_vb(1)

            # filler schedule: (block, group) -> emitter.  k slice s must land
            # before score group 2s; v batch b before PV step 2b+1; q_jb
            # before block jb; psob before block 0's tail (run at block1 g4).
            fillers = {
                (0, 0): lambda: emit_vb(2),
                (0, 1): lambda: (emit_k(4), emit_xb3()),
                (0, 2): lambda: emit_vb(3),
                (0, 4): lambda: emit_vb(4),
                (0, 5): lambda: emit_k(5),
                (0, 6): lambda: emit_vb(5),
                (0, 8): lambda: (emit_q(1), emit_k(6)),
                (0, 9): lambda: emit_vb(6),
                (0, 11): lambda: emit_k(7),
                (0, 12): lambda: emit_vb(7),
                (1, 0): emit_psob,
                (1, 1): lambda: emit_q(2),
                (1, 3): lambda: emit_q(3),
            }

            # deferred tail state: (pv, dn, dacc_fold, jb) of the previous block
            pend = [None]

            def tail_a():
                # den fold matmul + reciprocal of the previous block; if the
                # whole denominator accumulated on DVE this is dn's only
                # matmul and must open the psum accumulation group
                pv_p, dn_p, dfold_p, jb_p, den_g_p = pend[0]
                nc.tensor.matmul(
                    dn_p[:], lhsT=ones_bf[:], rhs=dfold_p[:],
                    start=(den_g_p == NG), stop=True,
                )
                rden = ostage.tile([128, NB], f32, tag="rden")
                nc.vector.reciprocal_approx_fast(out=rden[:], in_=dn_p[:])
                pend[0] = (pv_p, rden, jb_p)

            def tail_b(last=False):
                # PV drain, output projection, residual, store.  For the last
                # block the normalization moves AFTER the projection (divide
                # by den commutes with the channel matmul), so the PV drain
                # and projection overlap the denominator reciprocal.
                pv_p, rden, jb_p = pend[0]
                pend[0] = None
                hv = ostage.tile([C, NB], bf16, tag="hv")
                if last:
                    nc.vector.tensor_copy(out=hv[:], in_=pv_p[:])
                else:
                    nc.vector.tensor_tensor(hv[:], pv_p[:], rden[:], mult_op)
                pso = ps_m.tile([C, NB], f32, tag="m")
                nc.tensor.matmul(pso[:], lhsT=w_p[:], rhs=hv[:], start=True, stop=True)
                o1 = ostage.tile([C, NB], f32, tag="o1")
                xblk = xc[jb_p // 2][:, (jb_p % 2) * 512 : (jb_p % 2) * 512 + 512]
                if last:
                    on = ostage.tile([C, NB], f32, tag="on")
                    nc.vector.tensor_tensor(on[:], pso[:], rden[:], mult_op)
                    nc.vector.scalar_tensor_tensor(
                        o1[:], on[:], obs[:], xblk, op0=sub_op, op1=add_op
                    )
                else:
                    nc.vector.scalar_tensor_tensor(
                        o1[:], pso[:], obs[:], xblk, op0=sub_op, op1=add_op
                    )
                nc.sync.dma_start(
                    out=out_d[:, jb_p * NB : (jb_p + 1) * NB], in_=o1[:]
                )

            # --- attention over query blocks ---
            for jb in range(NBLK):
                den_g = DEN_SPLIT[jb]
                qs = qb[jb][:]
                pv = ps_pv.tile([C, NB], f32, tag="pv")
                dn = ps_den.tile([C, NB], f32, tag="dn")
                dacc = ostage.tile([128, EXP_GRP, NB], bf16, tag="dacc")
                dfold = ostage.tile([128, NB], bf16, tag="dfold")
                pts = [None] * NG
                # software-pipelined by one group: scores/exp for g are
                # emitted ahead of group g-1's consumers so the scalar engine
                # never starves behind PV/den matmuls.
                for g in range(NG + 1):
                    if g < NG:
                        ss = ps_s.tile([128, EXP_GRP, NB], f32, tag="s")
                        for u in range(EXP_GRP):
                            mi = g * EXP_GRP + u
                            nc.tensor.matmul(
                                ss[:, u, :],
                                lhsT=kpart(mi),
                                rhs=qs,
                                start=True,
                                stop=True,
                            )
                        pt = ptile.tile([128, EXP_GRP, NB], bf16, tag="pt")
                        nc.scalar.activation(
                            out=pt[:], in_=ss[:], func=Exp, scale=SCALE
                        )
                        pts[g] = pt
                        fill = fillers.get((jb, g))
                        if fill is not None:
                            fill()
                    if pend[0] is not None:
                        if g == 2:
                            tail_a()
                        elif g == 4:
                            tail_b()
                    if g == 0:
                        continue
                    c = g - 1
                    pt = pts[c]
                    pts[c] = None
                    for u in range(EXP_GRP):
                        mi = c * EXP_GRP + u
                        nc.tensor.matmul(
                            pv[:],
                            lhsT=vT_sb[:, mi, :],
                            rhs=pt[:, u, :],
                            start=(mi == 0),
                            stop=(mi == MT - 1),
                        )
                    if c < den_g:
                        # denominator partial on DVE (bf16 SBUF adds, 2x mode)
                        if c == 0:
                            nc.vector.tensor_copy(out=dacc[:], in_=pt[:])
                        else:
                            nc.vector.tensor_tensor(dacc[:], dacc[:], pt[:], add_op)
                        if c == den_g - 1:
                            nc.vector.tensor_tensor(
                                dfold[:], dacc[:, 0, :], dacc[:, 1, :], add_op
                            )
                    else:
                        # denominator partial on PE; the all-ones stationary
                        # writes den into every output partition, so the
                        # reciprocal needs no cross-partition broadcast
                        for u in range(EXP_GRP):
                            mi = c * EXP_GRP + u
                            nc.tensor.matmul(
                                dn[:],
                                lhsT=ones_bf[:],
                                rhs=pt[:, u, :],
                                start=(c == den_g and u == 0),
                                stop=False,
                            )
                pend[0] = (pv, dn, dfold, jb, den_g)
            # last block's tail runs immediately
            tail_a()
            tail_b(last=True)

    nc.compile()
    _NC_CACHE[key] = nc
    return nc


def kernel(**inputs):
    global LAST_RESULTS
    _install_ntff_hook()
    from concourse.bass_utils import run_bass_kernel_spmd

    ins = {
        k: np.ascontiguousarray(np.asarray(v), dtype=np.float32)
        for k, v in inputs.items()
    }
    x = ins["x"]
    gs, gb = ins["gn_scale"], ins["gn_bias"]

    # Fold the GroupNorm affine into the q/k/v weights; pre-transpose all
    # weights into the [in_channel, out_channel] layout the PE wants.
    wq_e = ins["wq"] * gs[None, :]
    wk_e = ins["wk"] * gs[None, :]
    wv_e = ins["wv"] * gs[None, :]
    wqT = np.ascontiguousarray(wq_e.T)
    wkT = np.ascontiguousarray(wk_e.T)
    wvT = np.ascontiguousarray(wv_e.T)
    wpT = np.ascontiguousarray(ins["wp"].T)
    bq_e = (ins["bq"] + ins["wq"] @ gb).reshape(C, 1)
    bv_e = ins["bv"] + ins["wv"] @ gb
    bp_e = (ins["bp"] + ins["wp"] @ bv_e).reshape(C, 1)
    use_bq = bool(np.any(bq_e))
    use_bp = bool(np.any(bp_e))

    nc = _build(use_bq, use_bp)

    in_maps = []
    for core in range(8):
        b, half = core // 2, core % 2
        xb = x[b].reshape(C, N)
        if half == 1:
            xb = np.concatenate([xb[:, NQ:], xb[:, :NQ]], axis=1)
        in_maps.append(
            {
                "xp": np.ascontiguousarray(xb),
                "wqT": wqT,
                "wkT": wkT,
                "wvT": wvT,
                "wpT": wpT,
                "bqe": bq_e,
                "bpe": bp_e,
            }
        )

    trace = os.environ.get("KERNEL_TRACE", "0") == "1"
    res = run_bass_kernel_spmd(nc, in_maps, core_ids=list(range(8)), trace=trace)
    LAST_RESULTS = res

    out = np.empty((B, C, N), np.float32)
    for core in range(8):
        b, half = core // 2, core % 2
        out[b, :, half * NQ : (half + 1) * NQ] = res.results[core]["out"]
    return out.reshape(B, C, H, W)
